# revision 9
# baseline (speedup 1.0000x reference)
"""Trainium2 Bass kernel for a 2-bit-quantized (DoReFa) ResNet BasicBlock.

Full (unsharded) numpy inputs -> full numpy output, 8 images/core on 8
NeuronCores (data parallel, weights/BN replicated).

v4 design (vs the 132.4us v3):
  Trace analysis of v3 showed the 98.2us real-matmul stream already runs
  gapless at ~1 column/cycle (the fp8-DoubleRow hardware peak; LDWEIGHTS
  fully overlaps). All recoverable time was at the edges:
    - 27.3us of startup before the first real matmul: the DMA engines
      were saturated loading 10.9MB, of which 3.7MB were constant-8.0
      plane fills (z8) whose interiors get fully overwritten by the
      quantization writes anyway.
    - ~7us of tail: the final co-phase's 8 y-DMA issues serialized on
      one queue (~600ns each) plus teardown.
  v4 therefore:
    - replaces the z8 DMA fills with 3 tiny pad-cell memsets per plane
      (87 of 871 cells actually need the 8.0 pad value) on the
      otherwise-idle GpSimd engine;
    - reorders startup: x[0], x[1] load first, their quantization
      mult-add runs on the Activation engine (DVE only does the clamps),
      w1 rides behind x[1], remaining images load/quant after;
    - trims the PE p-state warm-up to 28 dummy matmuls (bridges PE
      readiness ~7.4us to first-real-matmul readiness ~12.5us);
    - spreads the final co-phase y-DMA issues across sync+gpsimd queues.

v3 core structure (kept):
  - Quantized activations live as qa' = q + 8 in {8..11} (fp8e4m3). In
    [8,16) the fp8 grid spacing is exactly 1, so writing (3x + 8) with an
    fp8-output op performs round-to-nearest-integer in hardware (RNE ties
    match jnp.round). The +8 offset folds into per-channel epilogue
    constants via the weight tap-sums.
  - conv1 epilogue: ACT(psum, Identity, scale,bias)->fp8 rounds+quantizes
    in ONE scalar op; DVE clamps into the padded qa2 plane (pad = 8).
  - conv2 epilogue: scalar_tensor_tensor (psum*s2 + residual) then
    (add b2, max 0) on DVE.
  - shared-pad plane layout: row stride 29, one pad cell between rows
    serves as row r's right pad AND row r+1's left pad, so each matmul
    streams 405 psum columns (392 useful).
  - each 3x3 conv = 9 shifted fp8 DoubleRow matmuls, scheduled
    k-outermost over quads of 4 psum banks so one LDWEIGHTS serves 4
    matmuls; redundant LDWEIGHTS are deleted pre-compile.
  - weight quantization + BN folding on host (O(weights) work).
"""

import os
import sys
import numpy as np


def _install_ntff_hook_shim():
    """Provide antenv.axon_hooks if the image lacks it, so
    run_bass_kernel_spmd(trace=True) can capture NTFF profiles through
    libaxon_pjrt.so. No-op if the real module exists or the .so is absent."""
    try:
        import antenv.axon_hooks  # noqa: F401
        return
    except ImportError:
        pass
    import contextlib
    import ctypes
    import types

    so_path = "/opt/axon/libaxon_pjrt.so"
    _hook = None
    if os.path.exists(so_path):
        try:
            lib = ctypes.CDLL(so_path)
        except OSError:
            lib = None
        if lib is not None and hasattr(lib, "axon_start_nrt_profile"):
            lib.axon_start_nrt_profile.argtypes = [
                ctypes.POINTER(ctypes.c_int64), ctypes.c_size_t]
            lib.axon_start_nrt_profile.restype = ctypes.c_int64
            lib.axon_stop_nrt_profile.argtypes = [ctypes.c_char_p]
            lib.axon_stop_nrt_profile.restype = ctypes.c_int64

            @contextlib.contextmanager
            def _hook(output_dir, device_ids):
                import jax
                jax.devices()
                if device_ids:
                    ids = (ctypes.c_int64 * len(device_ids))(*device_ids)
                    rc = lib.axon_start_nrt_profile(ids, len(device_ids))
                else:
                    rc = lib.axon_start_nrt_profile(None, 0)
                if rc != 0:
                    raise RuntimeError(f"axon_start_nrt_profile rc={rc}")
                try:
                    yield
                finally:
                    n = lib.axon_stop_nrt_profile(str(output_dir).encode())
                    print(f"profile: {n} file(s) written to {output_dir}",
                          file=sys.stderr)

    mod = types.ModuleType("antenv.axon_hooks")
    mod.get_axon_ntff_profile_hook = lambda: _hook
    mod.set_axon_ntff_profile_hook = lambda h: None
    sys.modules["antenv.axon_hooks"] = mod


NCORES = 8
NPER = 8          # images per core
GSZ = 4           # images per pipeline group
C = 256
NCH = 2           # channel chunks of 128
H = W = 28
PIX = H * W
PW = 29           # shared-pad plane row stride (28 data + 1 shared pad)
PEXT = 1 + 30 * PW  # plane extent: leading pad cell + 30 rows
QSTR = 896        # allocated plane stride per chunk (16B aligned >= 871)
HALF = 14         # rows per psum tile
RUN = (HALF - 1) * PW + W   # 405-element flat moving-run per matmul
PSF = HALF * PW   # 406 psum columns (col 28 of each row ignored)
QOFF = 8.0        # qa' = q + 8; fp8e4m3 spacing in [8,16) is exactly 1
BN_EPS = 1e-5
NWARM = 25        # PE p-state warm-up matmuls (256 cols, ~213ns each)


def _quant_weight3(w):
    """Replicate reference _quant_weight in f32, scaled by 3 -> {-3,-1,1,3}."""
    w = np.asarray(w, np.float32)
    t = np.tanh(w)
    m = np.max(np.abs(t))
    t2 = t / (np.float32(2.0) * m) + np.float32(0.5)
    k = np.round(t2 * np.float32(3.0))          # round-half-even == jnp.round
    return (2.0 * k - 3.0).astype(np.float32)


def _fold_bn(g, b, m, v):
    inv = np.asarray(g, np.float64) / np.sqrt(np.asarray(v, np.float64) + BN_EPS)
    beta = np.asarray(b, np.float64) - np.asarray(m, np.float64) * inv
    return inv, beta


def _w_tiles(qw3, dt):
    # [O, I, 3, 3] -> [p=128, co=2, k=9, ci=2, oc=128]: lhsT slices are
    # [128, 2, 128] interleaved ci-chunks for fp8 DoubleRow, and each
    # co-half [p, 9, 2, 128] is a contiguous run for clean DMA halves.
    return np.ascontiguousarray(
        np.transpose(qw3.reshape(NCH, 128, NCH, 128, 9), (3, 0, 4, 2, 1))
    ).astype(dt)


def _perch(v):
    # [C] f64 -> [128, NCH] f32 laid out so o = co*128 + p
    return np.ascontiguousarray(
        np.asarray(v, np.float64).reshape(NCH, 128).T).astype(np.float32)


def _host_arrays(w1, g1, b1, m1, v1, w2, g2, b2, m2, v2):
    from concourse import mybir
    f8 = mybir.dt.np(mybir.dt.float8e4)
    qw3_1 = _quant_weight3(w1)
    qw3_2 = _quant_weight3(w2)
    inv1, beta1 = _fold_bn(g1, b1, m1, v1)
    inv2, beta2 = _fold_bn(g2, b2, m2, v2)
    k1f = qw3_1.reshape(C, -1).sum(axis=1).astype(np.float64)
    k2f = qw3_2.reshape(C, -1).sum(axis=1).astype(np.float64)

    # conv1 psum P1' = P1 + 8*K1f (inputs are q+8). Want fp8(3*y + 8) with
    # y = P1*inv1/9 + beta1:  3y+8 = P1'*(inv1/3) + (3*beta1 + 8 - 8/3*K1f*inv1)
    s1 = inv1 / 3.0
    bb1 = 3.0 * beta1 + QOFF - (QOFF / 3.0) * k1f * inv1
    # conv2: y2 = P2'*(inv2/9) + (beta2 - 8/9*K2f*inv2); out = relu(y2 + x)
    s2 = inv2 / 9.0
    bb2 = beta2 - (QOFF / 9.0) * k2f * inv2

    # single param tile: cols [s1(2) b1(2) s2(2) b2(2) c8(1)]
    ppar = np.concatenate(
        [_perch(s1), _perch(bb1), _perch(s2), _perch(bb2),
         np.full((128, 1), QOFF, np.float32)], axis=1)
    return {"w1t": _w_tiles(qw3_1, f8), "w2t": _w_tiles(qw3_2, f8),
            "ppar": np.ascontiguousarray(ppar)}


def _dedup_ldweights(nc):
    """Delete InstLdweights whose stationary AP matches the previous load
    on the PE stream (no intervening different load / PE barrier). The PE
    array keeps its stationary operand across matmuls, so these reloads
    are redundant; each costs ~139ns on the PE queue (overlapped, but
    they crowd the reorder window)."""
    from concourse import mybir
    removed = 0
    for fn in nc.m.functions:
        for blk in fn.blocks:
            out = []
            last_sig = None
            for inst in blk.instructions:
                tn = type(inst).__name__
                if tn == "InstLdweights":
                    si = inst.sync_info
                    clean = si is None or (
                        len(si.on_wait) == 0 and len(si.on_update) == 0)
                    sig = str(inst.ins)
                    if sig == last_sig and clean:
                        removed += 1
                        continue
                    last_sig = sig
                elif tn != "InstMatmult" and getattr(
                        inst, "engine", None) == mybir.EngineType.PE:
                    last_sig = None  # drains etc: be conservative
                out.append(inst)
            blk.instructions = out
    return removed


def _build_program(nper=NPER, stage=3):
    from concourse import bacc, tile, mybir
    dt = mybir.dt
    op = mybir.AluOpType
    AF = mybir.ActivationFunctionType
    DR = mybir.MatmulPerfMode.DoubleRow

    nc = bacc.Bacc("TRN2", target_bir_lowering=False, debug=False,
                   num_devices=NCORES)
    NP_ = nper

    x_d = nc.dram_tensor("x", [NP_, C, PIX], dt.float32, kind="ExternalInput")
    w1_d = nc.dram_tensor("w1t", [128, NCH, 9, NCH, 128], dt.float8e4,
                          kind="ExternalInput")
    w2_d = nc.dram_tensor("w2t", [128, NCH, 9, NCH, 128], dt.float8e4,
                          kind="ExternalInput")
    pp_d = nc.dram_tensor("ppar", [128, 9], dt.float32, kind="ExternalInput")
    y_d = nc.dram_tensor("y", [NP_, C, PIX], dt.float32, kind="ExternalOutput")

    with tile.TileContext(nc) as tc:
        with (
            tc.tile_pool(name="wpool", bufs=1) as wpool,
            tc.tile_pool(name="xpool", bufs=NP_) as xpool,
            tc.tile_pool(name="qpool", bufs=NP_) as qpool,
            tc.tile_pool(name="tpool", bufs=5) as tpool,
            tc.tile_pool(name="epool", bufs=12) as epool,
            tc.tile_pool(name="pspool", bufs=8, space="PSUM") as pspool,
        ):
            mpool = vpool = opool = tpool
            # [p, co, conv*9+k, ci, oc]
            w_sb = wpool.tile([128, NCH, 18, NCH, 128], dt.float8e4,
                              name="wsb")
            pp_sb = wpool.tile([128, 9], dt.float32, name="ppsb")

            def s1c(co):
                return pp_sb[:, 0 + co:1 + co]

            def b1c(co):
                return pp_sb[:, 2 + co:3 + co]

            def s2c(co):
                return pp_sb[:, 4 + co:5 + co]

            def b2c(co):
                return pp_sb[:, 6 + co:7 + co]

            c8_sb = pp_sb[:, 8:9]
            qa1 = [qpool.tile([128, NCH, QSTR], dt.float8e4, name=f"qa1_{n}",
                              tag="qa1") for n in range(NP_)]
            qa2 = [qpool.tile([128, NCH, QSTR], dt.float8e4, name=f"qa2_{n}",
                              tag="qa2") for n in range(NP_)]
            x_sb = [None] * NP_

            nc.sync.dma_start(pp_sb[:], pp_d[:])

            def pad_memsets(qa_t):
                # only the 87 pad cells of the 871-cell plane need the 8.0
                # fill; the 784 interior cells are fully overwritten by the
                # quantization (or conv1-epilogue) writes before any read.
                # top band: leading cell + top pad row -> cells [0, 30)
                nc.gpsimd.memset(qa_t[:, :, 0:PW + 1], QOFF)
                # shared L/R pads: cells {2*PW + PW*k : k=0..27}
                nc.gpsimd.memset(
                    qa_t[:, :, 2 * PW:2 * PW + 28 * PW].rearrange(
                        "p j (r c) -> p j r c", c=PW)[:, :, :, 0:1], QOFF)
                # bottom pad row: cells [1 + 29*PW, 871)
                nc.gpsimd.memset(qa_t[:, :, 1 + 29 * PW:PEXT], QOFF)

            def interior4(qa_t):
                # [128, NCH, 28, 28] view of the two plane interiors
                # (data row r lives at offset 1 + 29*(r+1))
                return qa_t[:, :, 1:1 + 29 * PW].rearrange(
                    "p j (r c) -> p j r c", c=PW)[:, :, 1:1 + H, 0:W]

            def interior3(qa_t, co, h):
                return qa_t[:, co, 1:1 + 29 * PW].rearrange(
                    "p (r c) -> p r c", c=PW)[:, 1 + h * HALF:
                                              1 + (h + 1) * HALF, 0:W]

            def interior3j(qa_t, j):
                return qa_t[:, j, 1:1 + 29 * PW].rearrange(
                    "p (r c) -> p r c", c=PW)[:, 1:1 + H, 0:W]

            def xq_load(n, quarters=False, c1eng=None):
                xt = xpool.tile([128, NCH, PIX], dt.float32,
                                name=f"x_{n}", tag="x")
                if quarters:
                    # halve each chunk across both hw-DGE rings so the
                    # image lands as early as either ring allows
                    hp = PIX // 2
                    for j in range(NCH):
                        nc.sync.dma_start(xt[:, j, 0:hp],
                                          x_d[n, j * 128:j * 128 + 128, 0:hp])
                        nc.scalar.dma_start(
                            xt[:, j, hp:PIX],
                            x_d[n, j * 128:j * 128 + 128, hp:PIX])
                else:
                    nc.sync.dma_start(xt[:, 0, :], x_d[n, 0:128, :])
                    (c1eng or nc.scalar).dma_start(
                        xt[:, 1, :], x_d[n, 128:C, :])
                x_sb[n] = xt

            def interior3j_rows(qa_t, j, r0, r1):
                return qa_t[:, j, 1:1 + 29 * PW].rearrange(
                    "p (r c) -> p r c", c=PW)[:, 1 + r0:1 + r1, 0:W]

            def xq_quant(n, use_act, split_rows=False):
                xt = x_sb[n]
                qrs = []
                for j in range(NCH):
                    qr = mpool.tile([128, PIX], dt.float8e4, name="qr",
                                    tag="qr")
                    if use_act:
                        # fp8 write rounds 3x+8 to the integer grid (RNE);
                        # Act engine keeps the DVE free for the clamps
                        nc.scalar.activation(qr[:], xt[:, j, :],
                                             AF.Identity,
                                             bias=c8_sb, scale=3.0)
                    else:
                        nc.vector.tensor_scalar(
                            qr[:], xt[:, j, :], 3.0, c8_sb,
                            op.mult, op.add)
                    qrs.append(qr[:].rearrange("p (r c) -> p r c", c=W))
                if split_rows:
                    # clamp rows 1..15 of both chunks first: the first h=0
                    # matmuls (plane rows 0..15) can then start before the
                    # bottom halves are in
                    for j in range(NCH):
                        nc.vector.tensor_scalar(
                            interior3j_rows(qa1[n], j, 0, 15),
                            qrs[j][:, 0:15, :],
                            QOFF, QOFF + 3.0, op.max, op.min)
                    for j in range(NCH):
                        nc.vector.tensor_scalar(
                            interior3j_rows(qa1[n], j, 15, H),
                            qrs[j][:, 15:H, :],
                            QOFF, QOFF + 3.0, op.max, op.min)
                else:
                    for j in range(NCH):
                        nc.vector.tensor_scalar(
                            interior3j(qa1[n], j), qrs[j],
                            QOFF, QOFF + 3.0, op.max, op.min)

            def psv(ps):
                return ps[:].rearrange("p (r c) -> p r c", c=PW)[:, :, 0:W]

            def conv_blk(kbase, qa, blk, co, pstiles):
                for k in range(9):
                    dy, dx = divmod(k, 3)
                    for (n, h) in blk:
                        off = (h * HALF + dy) * PW + dx
                        nc.tensor.matmul(
                            pstiles[(n, h)][:, 0:RUN],
                            w_sb[:, co, kbase + k, 0:NCH, :],
                            qa[n][:, 0:NCH, off:off + RUN],
                            start=(k == 0), stop=(k == 8), perf_mode=DR)

            def blocks_of(ns, width):
                quads = [[(n, h) for n in list(ns)[q * 2:q * 2 + 2]
                          for h in range(2)] for q in range(2)]
                if width == 2:
                    quads = [b[i:i + 2] for b in quads for i in (0, 2)]
                return quads

            def conv1_group(g):
                ns = range(g * GSZ, (g + 1) * GSZ)
                for co in range(NCH):
                    if g == 0 and co == 0:
                        # image 0's halves run alone so the very first
                        # matmuls need only its top clamp; then pairs
                        n0, n1, n2, n3 = ns
                        blks = [[(n0, 0)], [(n0, 1)], [(n1, 0), (n1, 1)],
                                [(n2, 0), (n2, 1)], [(n3, 0), (n3, 1)]]
                    else:
                        blks = blocks_of(ns, 4)
                    for blk in blks:
                        pstiles = {nh: pspool.tile([128, PSF], dt.float32,
                                                   name="ps1", tag="ps")
                                   for nh in blk}
                        conv_blk(0, qa1, blk, co, pstiles)
                        for (n, h) in blk:
                            e = epool.tile([128, HALF, W], dt.float8e4,
                                           name="e", tag="e")
                            # one op: scale+bias+round-to-quant-grid (fp8)
                            nc.scalar.activation(
                                e[:], psv(pstiles[(n, h)]), AF.Identity,
                                bias=b1c(co), scale=s1c(co))
                            nc.vector.tensor_scalar(
                                interior3(qa2[n], co, h), e[:],
                                QOFF, QOFF + 3.0, op.max, op.min)

            def conv2_group(g, final=False):
                ns = range(g * GSZ, (g + 1) * GSZ)
                for co in range(NCH):
                    if final and co == NCH - 1:
                        # the last image's halves run alone so only one
                        # tile's epilogue + y-DMA remains after the very
                        # last matmul
                        na, nb, nok, nz = ns
                        blks = [[(na, 0), (na, 1)], [(nb, 0), (nb, 1)],
                                [(nok, 0), (nok, 1)], [(nz, 0)], [(nz, 1)]]
                    else:
                        blks = blocks_of(ns, 4)
                    last_nh = blks[-1][-1]
                    for blk in blks:
                        pstiles = {nh: pspool.tile([128, PSF], dt.float32,
                                                   name="ps2", tag="ps")
                                   for nh in blk}
                        conv_blk(9, qa2, blk, co, pstiles)
                        for (n, h) in blk:
                            v = vpool.tile([128, HALF, W], dt.float32,
                                           name="v", tag="v")
                            xh = x_sb[n][:, co, :].rearrange(
                                "p (r c) -> p r c", c=W)[:, h * HALF:
                                                         (h + 1) * HALF, :]
                            nc.vector.scalar_tensor_tensor(
                                v[:], psv(pstiles[(n, h)]),
                                s2c(co), xh, op.mult, op.add)
                            o = opool.tile([128, HALF, W], dt.float32,
                                           name="o", tag="o")
                            nc.vector.tensor_scalar(
                                o[:], v[:], b2c(co), 0.0, op.add, op.max)
                            yt = y_d[n, co * 128:(co + 1) * 128,
                                     h * HALF * W:(h + 1) * HALF * W]
                            if final and (n, h) == last_nh:
                                # drain the very last tile on two queues
                                hh = HALF // 2
                                nc.sync.dma_start(
                                    yt[:, 0:hh * W], o[:, 0:hh, :])
                                nc.gpsimd.dma_start(
                                    yt[:, hh * W:HALF * W], o[:, hh:, :])
                            else:
                                # spread y-DMA issues (~600ns each) across
                                # queues so no phase is issue-bound
                                eng = nc.scalar if co == 0 else (
                                    nc.sync if h == 0 else nc.gpsimd)
                                eng.dma_start(yt, o[:])

            def dump_qa(qa):
                for n in range(NP_):
                    o = opool.tile([128, NCH, PIX], dt.float32, name="od",
                                   tag="od")
                    nc.vector.tensor_copy(o[:], interior4(qa[n]))
                    nc.sync.dma_start(
                        y_d[n].rearrange("(j p) q -> p j q", p=128), o[:])

            # PE warm-up: dummy matmuls during startup eat the p-state
            # clock ramp before the first real matmul (~12.5us readiness)
            wrm = wpool.tile([128, NCH, 256], dt.float8e4, name="wrm")
            nc.gpsimd.memset(wrm[:], 1.0)
            psw = pspool.tile([128, 256], dt.float32, name="psw", tag="ps")
            for _ in range(NWARM):
                nc.tensor.matmul(psw[:], wrm[:, :, 0:128], wrm[:, :, :],
                                 start=True, stop=True, perf_mode=DR)

            # startup: first quad's images load+quantize first, w1 rides
            # behind them, then the rest of the batch. Images 2..7's
            # second chunks ride the gpsimd software-DGE ring so their
            # issues never sit behind the Act-engine quant ops.
            xq_load(0, quarters=True)
            xq_load(1, quarters=True)
            for n in (0, 1):
                pad_memsets(qa1[n])
            nc.sync.dma_start(w_sb[:, 0, 0:9, :, :], w1_d[:, 0, :, :, :])
            nc.scalar.dma_start(w_sb[:, 1, 0:9, :, :], w1_d[:, 1, :, :, :])
            xq_quant(0, use_act=True, split_rows=True)
            xq_quant(1, use_act=True)
            for n in range(2, NP_):
                xq_load(n, c1eng=nc.gpsimd)
            for n in (2, 3):
                pad_memsets(qa1[n])
            for n in (0, 1, 2, 3):
                pad_memsets(qa2[n])
            for n in range(4, NP_):
                pad_memsets(qa1[n])
            for n in range(4, NP_):
                pad_memsets(qa2[n])
            for n in range(2, NP_):
                xq_quant(n, use_act=False)

            ngr = NP_ // GSZ
            for g in range(ngr):
                if stage >= 2:
                    conv1_group(g)
                if g == 0:
                    nc.sync.dma_start(w_sb[:, :, 9:18, :, :], w2_d[:])
                if stage >= 3 and g >= 1:
                    conv2_group(g - 1)
            if stage == 1:
                dump_qa(qa1)
            if stage == 2:
                dump_qa(qa2)
            if stage >= 3:
                conv2_group(ngr - 1, final=True)

    n_removed = _dedup_ldweights(nc)
    if os.environ.get("KERNEL_DEBUG"):
        print(f"dedup removed {n_removed} InstLdweights", file=sys.stderr)
    nc.compile()
    return nc


_CACHED = None


def _get_program():
    global _CACHED
    if _CACHED is None:
        _CACHED = _build_program(
            stage=int(os.environ.get("KERNEL_STAGE", "3")))
    return _CACHED


def kernel(x, w1, g1, b1, m1, v1, w2, g2, b2, m2, v2):
    _install_ntff_hook_shim()
    from concourse.bass_utils import run_bass_kernel_spmd

    x = np.asarray(x, np.float32)
    host = _host_arrays(w1, g1, b1, m1, v1, w2, g2, b2, m2, v2)

    xs = x.reshape(NCORES, NPER, C, PIX)
    in_maps = [{"x": np.ascontiguousarray(xs[c]), **host}
               for c in range(NCORES)]

    nc = _get_program()
    res = run_bass_kernel_spmd(
        nc, in_maps, core_ids=list(range(NCORES)),
        trace=bool(int(os.environ.get("KERNEL_TRACE", "0"))),
    )
    kernel.last_results = res
    y = np.concatenate([res.results[c]["y"][None] for c in range(NCORES)], 0)
    return np.ascontiguousarray(y.reshape(64, C, H, W).astype(np.float32))
